# revision 2
# baseline (speedup 1.0000x reference)
"""Trainium2 Bass kernel for nn_MultiLatentAttention (B=2, S=2048, E=1024, H=16, P=64).

Math (exact reassociation of the reference):
  q = (x@WQ)@proj_w + proj_b          ->  x @ (WQ@proj_w) + proj_b
  attn1 - lam*attn2                   ->  q' @ k^T with q' = [s*q1, -s*lam*q2]
  (q'k^T) v                           ->  q' @ (k^T v)      (linear attention, no softmax)
  heads @ result_weight               ->  base @ W_eff,  W_eff[p,e] = sum_h (h+1)*RW[h*64+p, e]

Sharding: 8 cores, token-parallel for q/base/out (512 tokens each).  k^T v needs a
full-batch reduction; collectives cost 50-80us on this runtime, so each core instead
computes k,v over its ENTIRE batch (x^T for the full batch is staged per-core, fp16,
with columns rotated so the core's own q-tokens are columns 0:512 -- k^T v is
permutation-invariant over tokens).
"""

import math

import numpy as np
import ml_dtypes

import concourse.bass as bass
import concourse.tile as tile
from concourse import mybir
from concourse.bass_utils import run_bass_kernel_spmd

E = 1024
H = 16
P = 64      # per-head width (latent/H)
B = 2
S = 2048
N_CORES = 8
SH = 512    # q-tokens per core
KO = E // 128   # 8 contraction chunks
TCH = S // 128  # 16 token chunks per batch
QCH = SH // 128  # 4 token chunks for q

F16 = mybir.dt.float16
F32 = mybir.dt.float32


def _fix_excess_waits(nc, keep=1):
    """Split instructions with >keep sem waits (this walrus rejects multi-wait Drains)."""
    n_fixed = 0
    for f in nc.m.functions:
        for bb in f.blocks:
            insts = bb.instructions
            i = 0
            while i < len(insts):
                inst = insts[i]
                si = inst.sync_info
                waits = list(si.on_wait) if si is not None else []
                if len(waits) > keep:
                    excess, kept = waits[:-keep], waits[-keep:]
                    inst.sync_info = mybir.SyncInfo(on_wait=kept, on_update=list(si.on_update))
                    for k, w in enumerate(excess):
                        ev = mybir.InstEventSemaphore(
                            name=nc.get_next_instruction_name(),
                            engine=inst.engine, ins=[], outs=[],
                            sync_info=mybir.SyncInfo(on_wait=[w], on_update=[]),
                        )
                        nc.register_instruction(ev)
                        insts.insert(i + k, ev)
                    i += len(excess)
                    n_fixed += 1
                i += 1
    return n_fixed


def build_bass():
    nc = bass.Bass(num_devices=N_CORES)
    xt = nc.declare_dram_parameter("xt", [128, KO, S], F16, isOutput=False)
    wkv = nc.declare_dram_parameter("wkv", [128, KO, 2 * P], F16, isOutput=False)
    wq = nc.declare_dram_parameter("wq", [128, KO, P], F16, isOutput=False)
    bkv = nc.declare_dram_parameter("bkv", [1, 2 * P], F16, isOutput=False)
    bq = nc.declare_dram_parameter("bq", [1, P], F16, isOutput=False)
    weff = nc.declare_dram_parameter("weff", [P, E], F16, isOutput=False)
    out = nc.declare_dram_parameter("out", [SH, E], F32, isOutput=True)

    with tile.TileContext(nc) as tc:
        with (
            tc.tile_pool(name="singles", bufs=1) as singles,
            tc.tile_pool(name="xtp", bufs=TCH) as xtp,
            tc.tile_pool(name="kvp", bufs=1) as kvp,
            tc.tile_pool(name="small", bufs=1) as small,
            tc.tile_pool(name="outp", bufs=3) as outp,
            tc.tile_pool(name="pskv", bufs=2, space="PSUM") as pskv,
            tc.tile_pool(name="psacc", bufs=1, space="PSUM") as psacc,
            tc.tile_pool(name="pso", bufs=2, space="PSUM") as pso,
        ):
            # ---- constants / weights ----
            wkv_sb = singles.tile([128, KO, 2 * P], F16)
            nc.sync.dma_start(out=wkv_sb, in_=wkv[:, :, :])
            wq_sb = singles.tile([128, KO, P], F16)
            nc.sync.dma_start(out=wq_sb, in_=wq[:, :, :])
            bkv_sb = singles.tile([1, 2 * P], F16)
            nc.sync.dma_start(out=bkv_sb, in_=bkv[:, :])
            bq_sb = singles.tile([1, P], F16)
            nc.sync.dma_start(out=bq_sb, in_=bq[:, :])
            weff_sb = singles.tile([P, E], F16)
            nc.sync.dma_start(out=weff_sb, in_=weff[:, :])
            ones = singles.tile([1, 128], F16)
            nc.vector.memset(ones, 1.0)

            # ---- stream xt in token chunks ----
            xt_tiles = []
            for i in range(TCH):
                t = xtp.tile([128, KO, 128], F16, tag="xt")
                nc.sync.dma_start(out=t, in_=xt[:, :, i * 128:(i + 1) * 128])
                xt_tiles.append(t)

            kv_sb = kvp.tile([128, TCH, 2 * P], F16)
            ps_m = psacc.tile([P, P], F32, tag="m")

            def kv_chunk(i):
                ps = pskv.tile([128, 2 * P], F32, tag="kv")
                for ko in range(KO):
                    nc.tensor.matmul(ps, xt_tiles[i][:, ko], wkv_sb[:, ko],
                                     start=(ko == 0), stop=False)
                nc.tensor.matmul(ps, ones, bkv_sb, start=False, stop=True)
                eng = nc.vector if i % 2 == 0 else nc.scalar
                if eng is nc.vector:
                    eng.tensor_copy(out=kv_sb[:, i], in_=ps)
                else:
                    eng.copy(out=kv_sb[:, i], in_=ps)
                # M partial: k-chunk^T @ v-chunk, accumulated across all chunks
                nc.tensor.matmul(ps_m, kv_sb[:, i, 0:P], kv_sb[:, i, P:2 * P],
                                 start=(i == 0), stop=(i == TCH - 1),
                                 skip_group_check=True)

            # kv for the first 4 chunks (the core's own q tokens)
            for i in range(QCH):
                kv_chunk(i)

            # qT = wq^T @ xt[:, :, 0:512] + bias  -> [P, SH]
            ps_q = psacc.tile([P, SH], F32, tag="q")
            for i in range(QCH):
                for ko in range(KO):
                    nc.tensor.matmul(ps_q[:, i * 128:(i + 1) * 128],
                                     wq_sb[:, ko], xt_tiles[i][:, ko],
                                     start=(ko == 0), stop=False,
                                     skip_group_check=True)
                nc.tensor.matmul(ps_q[:, i * 128:(i + 1) * 128],
                                 bq_sb, ones, start=False, stop=True,
                                 skip_group_check=True)
            qT_sb = small.tile([P, SH], F16)
            nc.scalar.copy(out=qT_sb, in_=ps_q)

            # kv + M for the remaining chunks
            for i in range(QCH, TCH):
                kv_chunk(i)

            m_sb = small.tile([P, P], F16)
            nc.vector.tensor_copy(out=m_sb, in_=ps_m)

            # baseT = M^T @ qT  -> [P, SH]
            ps_bt = psacc.tile([P, SH], F32, tag="bt")
            nc.tensor.matmul(ps_bt, m_sb, qT_sb, start=True, stop=True)
            bT_sb = small.tile([P, SH], F16)
            nc.vector.tensor_copy(out=bT_sb, in_=ps_bt)

            # out[t, :] = baseT[:, t]^T @ weff  (per 128-token chunk, two 512-col halves)
            for i in range(QCH):
                o_sb = outp.tile([128, E], F32, tag="o")
                for h in range(2):
                    ps = pso.tile([128, 512], F32, tag="po")
                    nc.tensor.matmul(ps, bT_sb[:, i * 128:(i + 1) * 128],
                                     weff_sb[:, h * 512:(h + 1) * 512],
                                     start=True, stop=True)
                    eng_v = (i + h) % 2 == 0
                    if eng_v:
                        nc.vector.tensor_copy(out=o_sb[:, h * 512:(h + 1) * 512], in_=ps)
                    else:
                        nc.scalar.copy(out=o_sb[:, h * 512:(h + 1) * 512], in_=ps)
                nc.sync.dma_start(out=out[i * 128:(i + 1) * 128, :], in_=o_sb)

    _fix_excess_waits(nc)
    return nc


def _host_prep(x, WQ, WK, WV, result_weight, proj_w, proj_b,
               q1_vector, k1_vector, q2_vector, k2_vector, lambda_init):
    f64 = np.float64
    scale = 1.0 / math.sqrt(E // H)
    lam = (math.exp(float(np.dot(q1_vector.astype(f64), k1_vector.astype(f64))))
           - math.exp(float(np.dot(q2_vector.astype(f64), k2_vector.astype(f64))))
           + float(lambda_init[0]))

    wq_eff = WQ.astype(f64) @ proj_w.astype(f64)   # [E, P]
    wk_eff = WK.astype(f64) @ proj_w.astype(f64)
    wv_eff = WV.astype(f64) @ proj_w.astype(f64)

    d = np.concatenate([np.full(P // 2, scale), np.full(P // 2, -scale * lam)])
    wq_s = wq_eff * d                                  # fold scale/lam into q weights
    bq_s = proj_b.astype(f64) * d

    mult = np.arange(1, H + 1, dtype=f64)
    weff = (result_weight.astype(f64).reshape(H, P, E) * mult[:, None, None]).sum(0)  # [P, E]

    wkv = np.concatenate([wk_eff, wv_eff], axis=1)     # [E, 2P]
    bkv = np.concatenate([proj_b.astype(f64), proj_b.astype(f64)])  # [2P]

    f16 = ml_dtypes.float16 if not hasattr(np, "float16") else np.float16
    def to16(a):
        return np.ascontiguousarray(a, dtype=np.float64).astype(np.float16)

    wkv16 = to16(wkv).reshape(KO, 128, 2 * P).transpose(1, 0, 2)   # [128, KO, 2P]
    wq16 = to16(wq_s).reshape(KO, 128, P).transpose(1, 0, 2)       # [128, KO, P]
    bkv16 = to16(bkv).reshape(1, 2 * P)
    bq16 = to16(bq_s).reshape(1, P)
    weff16 = to16(weff)                                             # [P, E]

    in_maps = []
    for c in range(N_CORES):
        b = c // (N_CORES // B)
        s0 = (c % (N_CORES // B)) * SH
        xT = np.ascontiguousarray(x[b].T)              # [E, S] f32
        xrot = np.concatenate([xT[:, s0:], xT[:, :s0]], axis=1) if s0 else xT
        xt16 = xrot.astype(np.float16).reshape(KO, 128, S).transpose(1, 0, 2)  # [128,KO,S]
        in_maps.append({
            "xt": np.ascontiguousarray(xt16),
            "wkv": np.ascontiguousarray(wkv16),
            "wq": np.ascontiguousarray(wq16),
            "bkv": bkv16,
            "bq": bq16,
            "weff": weff16,
        })
    return in_maps


_NC_CACHE = {}


def kernel(**inputs):
    inputs = {k: np.asarray(v) for k, v in inputs.items()}
    in_maps = _host_prep(**inputs)
    if "nc" not in _NC_CACHE:
        _NC_CACHE["nc"] = build_bass()
    res = run_bass_kernel_spmd(_NC_CACHE["nc"], in_maps, list(range(N_CORES)))
    out = np.empty((B, S, E), np.float32)
    for c in range(N_CORES):
        b = c // (N_CORES // B)
        s0 = (c % (N_CORES // B)) * SH
        out[b, s0:s0 + SH] = res.results[c]["out"]
    return out


# revision 4
# speedup vs baseline: 1.3620x; 1.3620x over previous
"""Trainium2 Bass kernel for nn_MultiLatentAttention (B=2, S=2048, E=1024, H=16, P=64).

Math (exact reassociation of the reference):
  q = (x@WQ)@proj_w + proj_b          ->  x @ (WQ@proj_w) + proj_b
  attn1 - lam*attn2                   ->  q' @ k^T with q' = [s*q1, -s*lam*q2]
  (q'k^T) v                           ->  q' @ (k^T v)      (linear attention, no softmax)
  heads @ result_weight               ->  base @ W_eff,  W_eff[p,e] = sum_h (h+1)*RW[h*64+p, e]

Sharding: 8 cores, token-parallel for q/base/out (512 tokens each).  k^T v needs a
full-batch reduction; collectives cost 50-80us on this runtime, so each core instead
computes k,v over its ENTIRE batch (x^T for the full batch is staged per-core, fp16,
with columns rotated so the core's own q-tokens are columns 0:512 -- k^T v is
permutation-invariant over tokens).
"""

import math

import numpy as np

import concourse.bass as bass
import concourse.tile as tile
from concourse import mybir
from concourse.bass_utils import run_bass_kernel_spmd

E = 1024
H = 16
P = 64        # per-head width (latent/H)
B = 2
S = 2048
N_CORES = 8
SH = 512      # q-tokens per core
KO = E // 128    # 8 contraction chunks
CH = 8           # xt DMA chunks (256 tokens each)
TPC = S // CH    # 256 tokens per DMA chunk
SUB = S // 128   # 16 compute sub-chunks of 128 tokens

F16 = mybir.dt.float16
F32 = mybir.dt.float32


def _fix_excess_waits(nc, keep=1):
    """Split instructions with >keep sem waits (this walrus rejects multi-wait Drains)."""
    n_fixed = 0
    for f in nc.m.functions:
        for bb in f.blocks:
            insts = bb.instructions
            i = 0
            while i < len(insts):
                inst = insts[i]
                si = inst.sync_info
                waits = list(si.on_wait) if si is not None else []
                if len(waits) > keep:
                    excess, kept = waits[:-keep], waits[-keep:]
                    inst.sync_info = mybir.SyncInfo(on_wait=kept, on_update=list(si.on_update))
                    for k, w in enumerate(excess):
                        ev = mybir.InstEventSemaphore(
                            name=nc.get_next_instruction_name(),
                            engine=inst.engine, ins=[], outs=[],
                            sync_info=mybir.SyncInfo(on_wait=[w], on_update=[]),
                        )
                        nc.register_instruction(ev)
                        insts.insert(i + k, ev)
                    i += len(excess)
                    n_fixed += 1
                i += 1
    return n_fixed


def build_bass():
    nc = bass.Bass(num_devices=N_CORES)
    # xt: [128(ki), CH, KO, TPC] -- per-partition contiguous per chunk
    xt = nc.declare_dram_parameter("xt", [128, CH, KO, TPC], F16, isOutput=False)
    # wkvq: fused [Wk|Wv|Wq'] -> [128(ki), KO, 192]
    wkvq = nc.declare_dram_parameter("wkvq", [128, KO, 3 * P], F16, isOutput=False)
    # rows: [bkv(128) | bq(64) | ones(256)]
    rows = nc.declare_dram_parameter("rows", [1, 448], F16, isOutput=False)
    weff = nc.declare_dram_parameter("weff", [P, E], F16, isOutput=False)
    out = nc.declare_dram_parameter("out", [SH, E], F32, isOutput=True)

    with tile.TileContext(nc) as tc:
        with (
            tc.tile_pool(name="singles", bufs=1) as singles,
            tc.tile_pool(name="xtp", bufs=CH) as xtp,
            tc.tile_pool(name="kvp", bufs=1) as kvp,
            tc.tile_pool(name="small", bufs=1) as small,
            tc.tile_pool(name="outp", bufs=3) as outp,
            tc.tile_pool(name="pskv", bufs=2, space="PSUM") as pskv,
            tc.tile_pool(name="psacc", bufs=1, space="PSUM") as psacc,
            tc.tile_pool(name="pso", bufs=2, space="PSUM") as pso,
        ):
            # ---- weights / constants (3 DMAs) ----
            wkvq_sb = singles.tile([128, KO, 3 * P], F16)
            nc.sync.dma_start(out=wkvq_sb, in_=wkvq[:, :, :])
            rows_sb = singles.tile([1, 448], F16)
            nc.sync.dma_start(out=rows_sb, in_=rows[:, :])
            bkv_sb = rows_sb[:, 0:128]
            bq_sb = rows_sb[:, 128:192]
            ones_sb = rows_sb[:, 192:448]

            # ---- stream xt in CH chunks of TPC tokens ----
            xt_tiles = []
            for i in range(CH):
                t = xtp.tile([128, KO, TPC], F16, tag="xt")
                nc.sync.dma_start(out=t, in_=xt[:, i])
                xt_tiles.append(t)

            weff_sb = singles.tile([P, E], F16)
            nc.sync.dma_start(out=weff_sb, in_=weff[:, :])

            kv_sb = kvp.tile([128, SUB, 2 * P], F16)

            # ---- k|v for every token sub-chunk (dense PE stream) ----
            def kv_chunk(j):
                i, half = j // 2, (j % 2) * 128
                ps = pskv.tile([128, 2 * P], F32, tag="kv")
                for ko in range(KO):
                    nc.tensor.matmul(ps, xt_tiles[i][:, ko, half:half + 128],
                                     wkvq_sb[:, ko, 0:2 * P],
                                     start=(ko == 0), stop=False)
                nc.tensor.matmul(ps, ones_sb[:, 0:128], bkv_sb, start=False, stop=True)
                if j % 2 == 0:
                    nc.vector.tensor_copy(out=kv_sb[:, j], in_=ps)
                else:
                    nc.scalar.copy(out=kv_sb[:, j], in_=ps)

            for j in range(4):
                kv_chunk(j)

            # ---- qT = wq^T @ xt[:, 0:512] + bq  -> [P, SH] (ko-outer, N=256) ----
            ps_q = psacc.tile([P, SH], F32, tag="q")
            for i in range(2):
                for ko in range(KO):
                    nc.tensor.matmul(ps_q[:, i * TPC:(i + 1) * TPC],
                                     wkvq_sb[:, ko, 2 * P:3 * P],
                                     xt_tiles[i][:, ko],
                                     start=(ko == 0), stop=False,
                                     skip_group_check=True)
                nc.tensor.matmul(ps_q[:, i * TPC:(i + 1) * TPC],
                                 bq_sb, ones_sb, start=False, stop=True,
                                 skip_group_check=True)
            qT_sb = small.tile([P, SH], F16)
            nc.scalar.copy(out=qT_sb, in_=ps_q)

            for j in range(4, SUB):
                kv_chunk(j)

            # ---- M = k^T v over all sub-chunks ----
            ps_m = psacc.tile([P, P], F32, tag="m")
            for j in range(SUB):
                nc.tensor.matmul(ps_m, kv_sb[:, j, 0:P], kv_sb[:, j, P:2 * P],
                                 start=(j == 0), stop=(j == SUB - 1),
                                 skip_group_check=True)
            m_sb = small.tile([P, P], F16)
            nc.vector.tensor_copy(out=m_sb, in_=ps_m)

            # ---- baseT = M^T @ qT  -> [P, SH] ----
            ps_bt = psacc.tile([P, SH], F32, tag="bt")
            nc.tensor.matmul(ps_bt, m_sb, qT_sb, start=True, stop=True)
            bT_sb = small.tile([P, SH], F16)
            nc.vector.tensor_copy(out=bT_sb, in_=ps_bt)

            # ---- out = baseT^T @ weff (4 token chunks x two 512-col halves) ----
            for i in range(SH // 128):
                o_sb = outp.tile([128, E], F32, tag="o")
                for h in range(2):
                    ps = pso.tile([128, 512], F32, tag="po")
                    nc.tensor.matmul(ps, bT_sb[:, i * 128:(i + 1) * 128],
                                     weff_sb[:, h * 512:(h + 1) * 512],
                                     start=True, stop=True)
                    if (i + h) % 2 == 0:
                        nc.vector.tensor_copy(out=o_sb[:, h * 512:(h + 1) * 512], in_=ps)
                    else:
                        nc.scalar.copy(out=o_sb[:, h * 512:(h + 1) * 512], in_=ps)
                nc.sync.dma_start(out=out[i * 128:(i + 1) * 128, :], in_=o_sb)

    _fix_excess_waits(nc)
    return nc


def _host_prep(x, WQ, WK, WV, result_weight, proj_w, proj_b,
               q1_vector, k1_vector, q2_vector, k2_vector, lambda_init):
    f64 = np.float64
    scale = 1.0 / math.sqrt(E // H)
    lam = (math.exp(float(np.dot(q1_vector.astype(f64), k1_vector.astype(f64))))
           - math.exp(float(np.dot(q2_vector.astype(f64), k2_vector.astype(f64))))
           + float(lambda_init[0]))

    wq_eff = WQ.astype(f64) @ proj_w.astype(f64)   # [E, P]
    wk_eff = WK.astype(f64) @ proj_w.astype(f64)
    wv_eff = WV.astype(f64) @ proj_w.astype(f64)

    d = np.concatenate([np.full(P // 2, scale), np.full(P // 2, -scale * lam)])
    wq_s = wq_eff * d
    bq_s = proj_b.astype(f64) * d

    mult = np.arange(1, H + 1, dtype=f64)
    weff = (result_weight.astype(f64).reshape(H, P, E) * mult[:, None, None]).sum(0)  # [P, E]

    wkvq = np.concatenate([wk_eff, wv_eff, wq_s], axis=1)          # [E, 3P]
    wkvq16 = wkvq.astype(np.float16).reshape(KO, 128, 3 * P).transpose(1, 0, 2)

    rows = np.zeros((1, 448), np.float16)
    rows[0, 0:P] = proj_b.astype(np.float16)
    rows[0, P:2 * P] = proj_b.astype(np.float16)
    rows[0, 2 * P:3 * P] = bq_s.astype(np.float16)
    rows[0, 192:448] = 1.0
    weff16 = weff.astype(np.float16)

    in_maps = []
    for c in range(N_CORES):
        b = c // (N_CORES // B)
        s0 = (c % (N_CORES // B)) * SH
        xT = x[b].T                                    # [E, S] f32 view
        xrot = np.concatenate([xT[:, s0:], xT[:, :s0]], axis=1) if s0 else xT
        # [ki, CH, KO, TPC]: e = ko*128 + ki, t = i*TPC + tt
        xt16 = (xrot.astype(np.float16)
                .reshape(KO, 128, CH, TPC)     # [ko, ki, i, tt]
                .transpose(1, 2, 0, 3))        # [ki, i, ko, tt]
        in_maps.append({
            "xt": np.ascontiguousarray(xt16),
            "wkvq": np.ascontiguousarray(wkvq16),
            "rows": rows,
            "weff": np.ascontiguousarray(weff16),
        })
    return in_maps


_NC_CACHE = {}


def kernel(**inputs):
    inputs = {k: np.asarray(v) for k, v in inputs.items()}
    in_maps = _host_prep(**inputs)
    if "nc" not in _NC_CACHE:
        _NC_CACHE["nc"] = build_bass()
    res = run_bass_kernel_spmd(_NC_CACHE["nc"], in_maps, list(range(N_CORES)))
    out = np.empty((B, S, E), np.float32)
    for c in range(N_CORES):
        b = c // (N_CORES // B)
        s0 = (c % (N_CORES // B)) * SH
        out[b, s0:s0 + SH] = res.results[c]["out"]
    return out


# revision 6
# speedup vs baseline: 1.3665x; 1.0033x over previous
"""Trainium2 Bass kernel for nn_MultiLatentAttention (B=2, S=2048, E=1024, H=16, P=64).

Math (exact reassociation of the reference):
  q = (x@WQ)@proj_w + proj_b          ->  x @ (WQ@proj_w) + proj_b
  attn1 - lam*attn2                   ->  q' @ k^T with q' = [s*q1, -s*lam*q2]
  (q'k^T) v                           ->  q' @ (k^T v)      (linear attention, no softmax)
  heads @ result_weight               ->  base @ W_eff,  W_eff[p,e] = sum_h (h+1)*RW[h*64+p, e]

Sharding: 8 cores, token-parallel for q/base/out (512 tokens each).  k^T v needs a
full-batch reduction; collectives cost 50-80us on this runtime, so each core instead
computes k,v over its ENTIRE batch (x^T for the full batch is staged per-core, fp16,
with columns rotated so the core's own q-tokens are columns 0:512 -- k^T v is
permutation-invariant over tokens).
"""

import math

import numpy as np

import concourse.bass as bass
import concourse.tile as tile
from concourse import mybir
from concourse.bass_utils import run_bass_kernel_spmd

E = 1024
H = 16
P = 64        # per-head width (latent/H)
B = 2
S = 2048
N_CORES = 8
SH = 512      # q-tokens per core
KO = E // 128    # 8 contraction chunks
CH = 8           # xt DMA chunks (256 tokens each)
TPC = S // CH    # 256 tokens per DMA chunk
SUB = S // 128   # 16 compute sub-chunks of 128 tokens

F16 = mybir.dt.float16
F32 = mybir.dt.float32


def _fix_excess_waits(nc, keep=1):
    """Split instructions with >keep sem waits (this walrus rejects multi-wait Drains)."""
    n_fixed = 0
    for f in nc.m.functions:
        for bb in f.blocks:
            insts = bb.instructions
            i = 0
            while i < len(insts):
                inst = insts[i]
                si = inst.sync_info
                waits = list(si.on_wait) if si is not None else []
                if len(waits) > keep:
                    excess, kept = waits[:-keep], waits[-keep:]
                    inst.sync_info = mybir.SyncInfo(on_wait=kept, on_update=list(si.on_update))
                    for k, w in enumerate(excess):
                        ev = mybir.InstEventSemaphore(
                            name=nc.get_next_instruction_name(),
                            engine=inst.engine, ins=[], outs=[],
                            sync_info=mybir.SyncInfo(on_wait=[w], on_update=[]),
                        )
                        nc.register_instruction(ev)
                        insts.insert(i + k, ev)
                    i += len(excess)
                    n_fixed += 1
                i += 1
    return n_fixed


def build_bass():
    nc = bass.Bass(num_devices=N_CORES)
    # xt: [128(ki), CH, KO, TPC] -- per-partition contiguous per chunk
    xt = nc.declare_dram_parameter("xt", [128, CH, KO, TPC], F16, isOutput=False)
    # wkvq: fused [Wk|Wv|Wq'] -> [128(ki), KO, 192]
    wkvq = nc.declare_dram_parameter("wkvq", [128, KO, 3 * P], F16, isOutput=False)
    # rows: [bkv(128) | bq(64) | ones(256)]
    rows = nc.declare_dram_parameter("rows", [1, 448], F16, isOutput=False)
    weff = nc.declare_dram_parameter("weff", [P, E], F16, isOutput=False)
    out = nc.declare_dram_parameter("out", [SH, E], F32, isOutput=True)

    with tile.TileContext(nc) as tc:
        with (
            tc.tile_pool(name="singles", bufs=1) as singles,
            tc.tile_pool(name="xtp", bufs=CH) as xtp,
            tc.tile_pool(name="kvp", bufs=1) as kvp,
            tc.tile_pool(name="small", bufs=1) as small,
            tc.tile_pool(name="outp", bufs=3) as outp,
            tc.tile_pool(name="pskv", bufs=2, space="PSUM") as pskv,
            tc.tile_pool(name="psacc", bufs=1, space="PSUM") as psacc,
            tc.tile_pool(name="pso", bufs=2, space="PSUM") as pso,
        ):
            # ---- first xt chunk, then weights, then the rest of xt ----
            xt_tiles = [None] * CH

            def load_chunk(i):
                t = xtp.tile([128, KO, TPC], F16, tag="xt")
                nc.sync.dma_start(out=t, in_=xt[:, i])
                xt_tiles[i] = t

            load_chunk(0)
            wkvq_sb = singles.tile([128, KO, 3 * P], F16)
            nc.sync.dma_start(out=wkvq_sb, in_=wkvq[:, :, :])
            rows_sb = singles.tile([1, 448], F16)
            nc.sync.dma_start(out=rows_sb, in_=rows[:, :])
            bkv_sb = rows_sb[:, 0:128]
            bq_sb = rows_sb[:, 128:192]
            ones_sb = rows_sb[:, 192:448]
            for i in range(1, CH):
                load_chunk(i)
            weff_sb = singles.tile([P, E], F16)
            nc.sync.dma_start(out=weff_sb, in_=weff[:, :])

            kv_sb = kvp.tile([128, SUB, 2 * P], F16)

            # ---- k|v for every token sub-chunk (dense PE stream) ----
            def kv_chunk(j):
                i, half = j // 2, (j % 2) * 128
                ps = pskv.tile([128, 2 * P], F32, tag="kv")
                for ko in range(KO):
                    nc.tensor.matmul(ps, xt_tiles[i][:, ko, half:half + 128],
                                     wkvq_sb[:, ko, 0:2 * P],
                                     start=(ko == 0), stop=False)
                nc.tensor.matmul(ps, ones_sb[:, 0:128], bkv_sb, start=False, stop=True)
                if j % 2 == 0:
                    nc.vector.tensor_copy(out=kv_sb[:, j], in_=ps)
                else:
                    nc.scalar.copy(out=kv_sb[:, j], in_=ps)

            for j in range(4):
                kv_chunk(j)

            # ---- qT = wq^T @ xt[:, 0:512] + bq  -> [P, SH] (ko-outer, N=256) ----
            ps_q = psacc.tile([P, SH], F32, tag="q")
            for i in range(2):
                for ko in range(KO):
                    nc.tensor.matmul(ps_q[:, i * TPC:(i + 1) * TPC],
                                     wkvq_sb[:, ko, 2 * P:3 * P],
                                     xt_tiles[i][:, ko],
                                     start=(ko == 0), stop=False,
                                     skip_group_check=True)
                nc.tensor.matmul(ps_q[:, i * TPC:(i + 1) * TPC],
                                 bq_sb, ones_sb, start=False, stop=True,
                                 skip_group_check=True)
            qT_sb = small.tile([P, SH], F16)
            nc.scalar.copy(out=qT_sb, in_=ps_q)

            # ---- M = k^T v, accumulated in groups of 4 sub-chunks ----
            ps_m = psacc.tile([P, P], F32, tag="m")

            def m_group(j0):
                for j in range(j0, j0 + 4):
                    nc.tensor.matmul(ps_m, kv_sb[:, j, 0:P], kv_sb[:, j, P:2 * P],
                                     start=(j == 0), stop=(j == SUB - 1),
                                     skip_group_check=True)

            m_group(0)
            for j in range(4, SUB):
                kv_chunk(j)
                if j % 4 == 3:
                    m_group(j - 3)

            m_sb = small.tile([P, P], F16)
            nc.vector.tensor_copy(out=m_sb, in_=ps_m)

            # ---- baseT = M^T @ qT  -> [P, SH] ----
            ps_bt = psacc.tile([P, SH], F32, tag="bt")
            nc.tensor.matmul(ps_bt, m_sb, qT_sb, start=True, stop=True)
            bT_sb = small.tile([P, SH], F16)
            nc.vector.tensor_copy(out=bT_sb, in_=ps_bt)

            # ---- out = baseT^T @ weff (4 token chunks x two 512-col halves) ----
            for i in range(SH // 128):
                o_sb = outp.tile([128, E], F32, tag="o")
                for h in range(2):
                    ps = pso.tile([128, 512], F32, tag="po")
                    nc.tensor.matmul(ps, bT_sb[:, i * 128:(i + 1) * 128],
                                     weff_sb[:, h * 512:(h + 1) * 512],
                                     start=True, stop=True)
                    if (i + h) % 2 == 0:
                        nc.vector.tensor_copy(out=o_sb[:, h * 512:(h + 1) * 512], in_=ps)
                    else:
                        nc.scalar.copy(out=o_sb[:, h * 512:(h + 1) * 512], in_=ps)
                nc.sync.dma_start(out=out[i * 128:(i + 1) * 128, :], in_=o_sb)

    _fix_excess_waits(nc)
    return nc


def _host_prep(x, WQ, WK, WV, result_weight, proj_w, proj_b,
               q1_vector, k1_vector, q2_vector, k2_vector, lambda_init):
    f64 = np.float64
    scale = 1.0 / math.sqrt(E // H)
    lam = (math.exp(float(np.dot(q1_vector.astype(f64), k1_vector.astype(f64))))
           - math.exp(float(np.dot(q2_vector.astype(f64), k2_vector.astype(f64))))
           + float(lambda_init[0]))

    wq_eff = WQ.astype(f64) @ proj_w.astype(f64)   # [E, P]
    wk_eff = WK.astype(f64) @ proj_w.astype(f64)
    wv_eff = WV.astype(f64) @ proj_w.astype(f64)

    d = np.concatenate([np.full(P // 2, scale), np.full(P // 2, -scale * lam)])
    wq_s = wq_eff * d
    bq_s = proj_b.astype(f64) * d

    mult = np.arange(1, H + 1, dtype=f64)
    weff = (result_weight.astype(f64).reshape(H, P, E) * mult[:, None, None]).sum(0)  # [P, E]

    wkvq = np.concatenate([wk_eff, wv_eff, wq_s], axis=1)          # [E, 3P]
    wkvq16 = wkvq.astype(np.float16).reshape(KO, 128, 3 * P).transpose(1, 0, 2)

    rows = np.zeros((1, 448), np.float16)
    rows[0, 0:P] = proj_b.astype(np.float16)
    rows[0, P:2 * P] = proj_b.astype(np.float16)
    rows[0, 2 * P:3 * P] = bq_s.astype(np.float16)
    rows[0, 192:448] = 1.0
    weff16 = weff.astype(np.float16)

    in_maps = []
    for c in range(N_CORES):
        b = c // (N_CORES // B)
        s0 = (c % (N_CORES // B)) * SH
        xT = x[b].T                                    # [E, S] f32 view
        xrot = np.concatenate([xT[:, s0:], xT[:, :s0]], axis=1) if s0 else xT
        # [ki, CH, KO, TPC]: e = ko*128 + ki, t = i*TPC + tt
        xt16 = (xrot.astype(np.float16)
                .reshape(KO, 128, CH, TPC)     # [ko, ki, i, tt]
                .transpose(1, 2, 0, 3))        # [ki, i, ko, tt]
        in_maps.append({
            "xt": np.ascontiguousarray(xt16),
            "wkvq": np.ascontiguousarray(wkvq16),
            "rows": rows,
            "weff": np.ascontiguousarray(weff16),
        })
    return in_maps


_NC_CACHE = {}


def kernel(**inputs):
    inputs = {k: np.asarray(v) for k, v in inputs.items()}
    in_maps = _host_prep(**inputs)
    if "nc" not in _NC_CACHE:
        _NC_CACHE["nc"] = build_bass()
    res = run_bass_kernel_spmd(_NC_CACHE["nc"], in_maps, list(range(N_CORES)))
    out = np.empty((B, S, E), np.float32)
    for c in range(N_CORES):
        b = c // (N_CORES // B)
        s0 = (c % (N_CORES // B)) * SH
        out[b, s0:s0 + SH] = res.results[c]["out"]
    return out


# revision 8
# speedup vs baseline: 1.3783x; 1.0087x over previous
"""Trainium2 Bass kernel for nn_MultiLatentAttention (B=2, S=2048, E=1024, H=16, P=64).

Math (exact reassociation of the reference):
  q = (x@WQ)@proj_w + proj_b          ->  x @ (WQ@proj_w) + proj_b
  attn1 - lam*attn2                   ->  q' @ k^T with q' = [s*q1, -s*lam*q2]
  (q'k^T) v                           ->  q' @ (k^T v)      (linear attention, no softmax)
  heads @ result_weight               ->  base @ W_eff,  W_eff[p,e] = sum_h (h+1)*RW[h*64+p, e]

Sharding: 8 cores, token-parallel for q/base/out (512 tokens each).  k^T v needs a
full-batch reduction; collectives cost 50-80us on this runtime, so each core instead
computes k,v over its ENTIRE batch (x^T for the full batch is staged per-core, fp16,
with columns rotated so the core's own q-tokens are columns 0:512 -- k^T v is
permutation-invariant over tokens).
"""

import math

import numpy as np

import concourse.bass as bass
import concourse.tile as tile
from concourse import mybir
from concourse.bass_utils import run_bass_kernel_spmd

E = 1024
H = 16
P = 64        # per-head width (latent/H)
B = 2
S = 2048
N_CORES = 8
SH = 512      # q-tokens per core
KO = E // 128    # 8 contraction chunks
CH = 8           # xt DMA chunks (256 tokens each)
TPC = S // CH    # 256 tokens per DMA chunk
SUB = S // 128   # 16 compute sub-chunks of 128 tokens

F16 = mybir.dt.float16
F32 = mybir.dt.float32


def _fix_excess_waits(nc, keep=1):
    """Split instructions with >keep sem waits (this walrus rejects multi-wait Drains)."""
    n_fixed = 0
    for f in nc.m.functions:
        for bb in f.blocks:
            insts = bb.instructions
            i = 0
            while i < len(insts):
                inst = insts[i]
                si = inst.sync_info
                waits = list(si.on_wait) if si is not None else []
                if len(waits) > keep:
                    excess, kept = waits[:-keep], waits[-keep:]
                    inst.sync_info = mybir.SyncInfo(on_wait=kept, on_update=list(si.on_update))
                    for k, w in enumerate(excess):
                        ev = mybir.InstEventSemaphore(
                            name=nc.get_next_instruction_name(),
                            engine=inst.engine, ins=[], outs=[],
                            sync_info=mybir.SyncInfo(on_wait=[w], on_update=[]),
                        )
                        nc.register_instruction(ev)
                        insts.insert(i + k, ev)
                    i += len(excess)
                    n_fixed += 1
                i += 1
    return n_fixed


def build_bass():
    nc = bass.Bass(num_devices=N_CORES)
    # xt: [128(ki), CH, KO, TPC] -- per-partition contiguous per chunk
    xt = nc.declare_dram_parameter("xt", [128, CH, KO, TPC], F16, isOutput=False)
    # wkvq: fused [Wk|Wv|Wq'] -> [128(ki), KO, 192]
    wkvq = nc.declare_dram_parameter("wkvq", [128, KO, 3 * P], F16, isOutput=False)
    # rows: [bkv(128) | bq(64) | ones(256)]
    rows = nc.declare_dram_parameter("rows", [1, 448], F16, isOutput=False)
    weff = nc.declare_dram_parameter("weff", [P, E + 1], F16, isOutput=False)
    out = nc.declare_dram_parameter("out", [SH, E], F32, isOutput=True)

    with tile.TileContext(nc) as tc:
        with (
            tc.tile_pool(name="singles", bufs=1) as singles,
            tc.tile_pool(name="xtp", bufs=CH) as xtp,
            tc.tile_pool(name="kvp", bufs=1) as kvp,
            tc.tile_pool(name="small", bufs=1) as small,
            tc.tile_pool(name="outp", bufs=3) as outp,
            tc.tile_pool(name="pskv", bufs=3, space="PSUM") as pskv,
            tc.tile_pool(name="psacc", bufs=1, space="PSUM") as psacc,
            tc.tile_pool(name="pso", bufs=3, space="PSUM") as pso,
        ):
            # ---- first xt chunk, then weights, then the rest of xt ----
            xt_tiles = [None] * CH

            def load_chunk(i, split=False):
                t = xtp.tile([128, KO, TPC], F16, tag="xt")
                if split:
                    nc.sync.dma_start(out=t[:, :, 0:128], in_=xt[:, i, :, 0:128])
                    nc.sync.dma_start(out=t[:, :, 128:TPC], in_=xt[:, i, :, 128:TPC])
                else:
                    nc.sync.dma_start(out=t, in_=xt[:, i])
                xt_tiles[i] = t

            load_chunk(0, split=True)
            wkvq_sb = singles.tile([128, KO, 3 * P], F16)
            nc.sync.dma_start(out=wkvq_sb, in_=wkvq[:, :, :])
            rows_sb = singles.tile([1, 448], F16)
            nc.sync.dma_start(out=rows_sb, in_=rows[:, :])
            bkv_sb = rows_sb[:, 0:128]
            bq_sb = rows_sb[:, 128:192]
            ones_sb = rows_sb[:, 192:448]
            for i in range(1, CH):
                load_chunk(i)
            weff_sb = singles.tile([P, E + 1], F16)
            nc.sync.dma_start(out=weff_sb, in_=weff[:, :])

            kv_sb = kvp.tile([128, SUB, 2 * P], F16)

            # ---- k|v for every token sub-chunk (dense PE stream) ----
            def kv_chunk(j):
                i, half = j // 2, (j % 2) * 128
                ps = pskv.tile([128, 2 * P], F32, tag="kv")
                for ko in range(KO):
                    nc.tensor.matmul(ps, xt_tiles[i][:, ko, half:half + 128],
                                     wkvq_sb[:, ko, 0:2 * P],
                                     start=(ko == 0), stop=False)
                nc.tensor.matmul(ps, ones_sb[:, 0:128], bkv_sb, start=False, stop=True)
                if j % 2 == 0:
                    nc.vector.tensor_copy(out=kv_sb[:, j], in_=ps)
                else:
                    nc.scalar.copy(out=kv_sb[:, j], in_=ps)

            for j in range(4):
                kv_chunk(j)

            # ---- qT = wq^T @ xt[:, 0:512] + bq  -> [P, SH] (ko-outer, N=256) ----
            ps_q = psacc.tile([P, SH], F32, tag="q")
            for i in range(2):
                for ko in range(KO):
                    nc.tensor.matmul(ps_q[:, i * TPC:(i + 1) * TPC],
                                     wkvq_sb[:, ko, 2 * P:3 * P],
                                     xt_tiles[i][:, ko],
                                     start=(ko == 0), stop=(ko == KO - 1),
                                     skip_group_check=True)
            qT_sb = small.tile([P, SH], F16)
            nc.scalar.activation(out=qT_sb, in_=ps_q,
                                 func=mybir.ActivationFunctionType.Identity,
                                 bias=weff_sb[:, E:E + 1])

            # ---- M = k^T v, accumulated in groups of 4 sub-chunks ----
            ps_m = psacc.tile([P, P], F32, tag="m")

            def m_group(j0):
                for j in range(j0, j0 + 4):
                    nc.tensor.matmul(ps_m, kv_sb[:, j, 0:P], kv_sb[:, j, P:2 * P],
                                     start=(j == 0), stop=(j == SUB - 1),
                                     skip_group_check=True)

            m_group(0)
            for j in range(4, SUB):
                kv_chunk(j)
                if j % 4 == 3:
                    m_group(j - 3)

            m_sb = small.tile([P, P], F16)
            nc.vector.tensor_copy(out=m_sb, in_=ps_m)

            # ---- baseT = M^T @ qT  -> [P, SH] ----
            ps_bt = psacc.tile([P, SH], F32, tag="q")
            nc.tensor.matmul(ps_bt, m_sb, qT_sb, start=True, stop=True)
            bT_sb = small.tile([P, SH], F16)
            nc.vector.tensor_copy(out=bT_sb, in_=ps_bt)

            # ---- out = baseT^T @ weff (4 token chunks x two 512-col halves) ----
            for i in range(SH // 128):
                o_sb = outp.tile([128, E], F32, tag="o")
                for h in range(2):
                    ps = pso.tile([128, 512], F32, tag="po")
                    nc.tensor.matmul(ps, bT_sb[:, i * 128:(i + 1) * 128],
                                     weff_sb[:, h * 512:(h + 1) * 512],
                                     start=True, stop=True)
                    if (i + h) % 2 == 0:
                        nc.vector.tensor_copy(out=o_sb[:, h * 512:(h + 1) * 512], in_=ps)
                    else:
                        nc.scalar.copy(out=o_sb[:, h * 512:(h + 1) * 512], in_=ps)
                nc.sync.dma_start(out=out[i * 128:(i + 1) * 128, :], in_=o_sb)

    _fix_excess_waits(nc)
    return nc


def _host_prep(x, WQ, WK, WV, result_weight, proj_w, proj_b,
               q1_vector, k1_vector, q2_vector, k2_vector, lambda_init):
    f64 = np.float64
    scale = 1.0 / math.sqrt(E // H)
    lam = (math.exp(float(np.dot(q1_vector.astype(f64), k1_vector.astype(f64))))
           - math.exp(float(np.dot(q2_vector.astype(f64), k2_vector.astype(f64))))
           + float(lambda_init[0]))

    wq_eff = WQ.astype(f64) @ proj_w.astype(f64)   # [E, P]
    wk_eff = WK.astype(f64) @ proj_w.astype(f64)
    wv_eff = WV.astype(f64) @ proj_w.astype(f64)

    d = np.concatenate([np.full(P // 2, scale), np.full(P // 2, -scale * lam)])
    wq_s = wq_eff * d
    bq_s = proj_b.astype(f64) * d

    mult = np.arange(1, H + 1, dtype=f64)
    weff = (result_weight.astype(f64).reshape(H, P, E) * mult[:, None, None]).sum(0)  # [P, E]

    wkvq = np.concatenate([wk_eff, wv_eff, wq_s], axis=1)          # [E, 3P]
    wkvq16 = wkvq.astype(np.float16).reshape(KO, 128, 3 * P).transpose(1, 0, 2)

    rows = np.zeros((1, 448), np.float16)
    rows[0, 0:P] = proj_b.astype(np.float16)
    rows[0, P:2 * P] = proj_b.astype(np.float16)
    rows[0, 192:448] = 1.0
    weff16 = np.concatenate([weff, bq_s[:, None]], axis=1).astype(np.float16)  # [P, E+1]

    in_maps = []
    for c in range(N_CORES):
        b = c // (N_CORES // B)
        s0 = (c % (N_CORES // B)) * SH
        xT = x[b].T                                    # [E, S] f32 view
        xrot = np.concatenate([xT[:, s0:], xT[:, :s0]], axis=1) if s0 else xT
        # [ki, CH, KO, TPC]: e = ko*128 + ki, t = i*TPC + tt
        xt16 = (xrot.astype(np.float16)
                .reshape(KO, 128, CH, TPC)     # [ko, ki, i, tt]
                .transpose(1, 2, 0, 3))        # [ki, i, ko, tt]
        in_maps.append({
            "xt": np.ascontiguousarray(xt16),
            "wkvq": np.ascontiguousarray(wkvq16),
            "rows": rows,
            "weff": np.ascontiguousarray(weff16),
        })
    return in_maps


_NC_CACHE = {}


def kernel(**inputs):
    inputs = {k: np.asarray(v) for k, v in inputs.items()}
    in_maps = _host_prep(**inputs)
    if "nc" not in _NC_CACHE:
        _NC_CACHE["nc"] = build_bass()
    res = run_bass_kernel_spmd(_NC_CACHE["nc"], in_maps, list(range(N_CORES)))
    out = np.empty((B, S, E), np.float32)
    for c in range(N_CORES):
        b = c // (N_CORES // B)
        s0 = (c % (N_CORES // B)) * SH
        out[b, s0:s0 + SH] = res.results[c]["out"]
    return out
